# revision 27
# baseline (speedup 1.0000x reference)
"""Converse2D (FFT-based closed-form deconvolution solve) on 8 Trainium2 cores.

v3 design:
  - Batch-packed forward FFT: x01 = x[2p] + i*x[2p+1] shares one complex
    128x128 FFT between two batches (forward matmul columns 768 -> 512
    per real slice).
  - Parity-separated inverse: Z_ab = X01 * T_ab per output parity ab;
    out[2p] = Re(ifft(Z_ab)), out[2p+1] = Im(ifft(Z_ab)).
  - stageA (row ifft) data-stationary; stageB (col ifft) flipped to
    constant-stationary form (stat = Fr/Fi/-Fi), 4 matmuls of N=512 per
    pair with a strided moving operand.
  - Software pipelining: forward+pointwise of channel ci is emitted ahead
    of the inverse of channel ci-1 so the DVE pointwise chain overlaps
    the PE's inverse matmuls.
  - T stored per parity as [Tr | Ti | Tr] so both pointwise products
    ([Tr|Ti] and [Ti|Tr]) read contiguous 256-blocks; all PSUM->SBUF
    copies have contiguous destinations.
  - bf16 output tiles (halves the output DMA; host converts to f32).

Sharding: core k handles channels [8k, 8k+8), all 4 batches (transfer
functions are reused across the batch).
"""

import numpy as np
import ml_dtypes

import concourse.bass as bass
import concourse.bacc as bacc
import concourse.mybir as mybir
import concourse.tile as tile
from concourse.bass_utils import run_bass_kernel_spmd

BF16 = ml_dtypes.bfloat16

B, C, H, W, KK = 4, 64, 128, 128, 5
S = 2
HS, WS = H * S, W * S
NCORES = 8
CPC = C // NCORES  # channels per core
NP = 2  # batch pairs per channel


# ----------------------------------------------------------------------------
# host-side precompute of the per-parity transfer functions
# ----------------------------------------------------------------------------
def _precompute_tc(weight: np.ndarray, lam: float) -> np.ndarray:
    """-> [C, 128, 1536] bf16: per parity ab=2a+b, blocks [Tr | Ti | Tr]."""
    psf = np.asarray(weight, np.float64)[0]  # [C,5,5]
    otf = np.zeros((C, HS, WS), np.complex128)
    otf[:, :KK, :KK] = psf
    otf = np.roll(otf, (-(KK // 2), -(KK // 2)), axis=(-2, -1))
    FB = np.fft.fft2(otf)
    FBC = np.conj(FB)
    F2B = (FB * FBC).real
    u = np.arange(HS)
    du = 1.0 + np.exp(-2j * np.pi * u / HS)
    Gbig = FBC + lam * du[:, None] * du[None, :]

    def quad_mean(A):
        return 0.25 * (A[:, :H, :W] + A[:, H:, :W] + A[:, :H, W:] + A[:, H:, W:])

    M = quad_mean(FB * Gbig) / (quad_mean(F2B) + lam)
    T = (Gbig - FBC * np.tile(M, (1, 2, 2))) / lam  # Tbig [C,256,256]

    ph = np.exp(2j * np.pi * np.arange(H) / HS)
    scale = 1.0 / (HS * WS)  # folds the 256-point ifft2 normalization
    blocks = []
    for a in range(2):
        for b in range(2):
            acc = np.zeros((C, H, W), np.complex128)
            for be in range(2):
                for ga in range(2):
                    acc += ((-1) ** (a * be + b * ga)) * T[
                        :, be * H : (be + 1) * H, ga * W : (ga + 1) * W
                    ]
            Tab = scale * (ph[:, None] ** a) * (ph[None, :] ** b) * acc
            Tr, Ti = Tab.real, Tab.imag
            blocks += [Tr, Ti - Tr, Ti + Tr]
    out = np.concatenate(blocks, axis=-1)  # [C,128,12*128=1536]
    return np.asarray(out, np.float32).astype(BF16)


# ----------------------------------------------------------------------------
# device program (built once, SPMD across 8 cores)
# ----------------------------------------------------------------------------
_CACHED_NC = None


def _build_nc():
    global _CACHED_NC
    if _CACHED_NC is not None:
        return _CACHED_NC

    f32 = mybir.dt.float32
    bf16 = mybir.dt.bfloat16

    idx = np.arange(H)
    Fc = np.exp(-2j * np.pi * np.outer(idx, idx) / H)
    Fr = Fc.real.astype(np.float32)
    Fi = Fc.imag.astype(np.float32)
    # forward rhs:  CF = [Fr | Fi],  CF2 = [-Fi | Fr]
    # inverse (G = conj(F) = Fr - i*Fi): CG = [Fr | -Fi], CG2 = [Fi | Fr]
    CF = np.concatenate([Fr, Fi], axis=1).astype(BF16)
    CF2 = np.concatenate([-Fi, Fr], axis=1).astype(BF16)
    CG = np.concatenate([Fr, -Fi], axis=1).astype(BF16)
    CG2 = np.concatenate([Fi, Fr], axis=1).astype(BF16)

    nc = bacc.Bacc()
    xs_ext = nc.dram_tensor("xs", [CPC, H, B * W], bf16, kind="ExternalInput")
    tc_ext = nc.dram_tensor("tc", [CPC, H, 1536], bf16, kind="ExternalInput")
    bias_ext = nc.dram_tensor("bias", [128, CPC], f32, kind="ExternalInput")
    out_ext = nc.dram_tensor("out", [CPC, NP, H, 1024], bf16, kind="ExternalOutput")

    cf_d = nc.inline_tensor(CF, "cf_d")
    cf2_d = nc.inline_tensor(CF2, "cf2_d")
    cg_d = nc.inline_tensor(CG, "cg_d")
    cg2_d = nc.inline_tensor(CG2, "cg2_d")

    with tile.TileContext(nc) as tc:
        from contextlib import ExitStack

        with ExitStack() as ctx:
            consts = ctx.enter_context(tc.tile_pool(name="consts", bufs=1))
            xpool = ctx.enter_context(tc.tile_pool(name="xpool", bufs=8))
            tpool = ctx.enter_context(tc.tile_pool(name="tpool", bufs=8))
            a1pool = ctx.enter_context(tc.tile_pool(name="a1pool", bufs=8))
            xspool = ctx.enter_context(tc.tile_pool(name="xspool", bufs=8))
            pppool = ctx.enter_context(tc.tile_pool(name="pppool", bufs=4))
            zpool = ctx.enter_context(tc.tile_pool(name="zpool", bufs=8))
            bspool = ctx.enter_context(tc.tile_pool(name="bspool", bufs=16))
            otpool = ctx.enter_context(tc.tile_pool(name="otpool", bufs=16))
            psum = ctx.enter_context(tc.tile_pool(name="psum", bufs=2, space="PSUM"))

            cf = consts.tile([128, 256], bf16, tag="cf")
            cf2 = consts.tile([128, 256], bf16, tag="cf2")
            cg = consts.tile([128, 256], bf16, tag="cg")
            cg2 = consts.tile([128, 256], bf16, tag="cg2")
            bias_t = consts.tile([128, CPC], f32, tag="bias")
            nc.sync.dma_start(cf[:], cf_d[:])
            nc.sync.dma_start(cf2[:], cf2_d[:])
            nc.sync.dma_start(cg[:], cg_d[:])
            nc.sync.dma_start(cg2[:], cg2_d[:])
            nc.sync.dma_start(bias_t[:], bias_ext[:])

            # stageB constant stationaries (slices of forward/inverse consts):
            #   Fr = cf[:,0:128], Fi = cf[:,128:256], Fin = -Fi = cg[:,128:256]
            st_fr = cf[:, 0:128]
            st_fi = cf[:, 128:256]
            st_fin = cg[:, 128:256]

            def fwd_s1(ci, st):
                """input DMAs + stage1 + a1 copy."""
                xt = xpool.tile([128, B * W], bf16)
                nc.gpsimd.dma_start(xt[:], xs_ext[ci])
                tt = tpool.tile([128, 1536], bf16)
                nc.gpsimd.dma_start(tt[:], tc_ext[ci])

                # stage1: pa[j, (p,c,u)] ; pair p packs batches 2p, 2p+1
                pa = psum.tile([128, 512], f32, tag="fwd", bufs=2)
                for p in range(NP):
                    reg = pa[:, 256 * p : 256 * p + 256]
                    nc.tensor.matmul(
                        reg, xt[:, 256 * p : 256 * p + 128], cf[:],
                        start=True, stop=False,
                    )
                    nc.tensor.matmul(
                        reg, xt[:, 256 * p + 128 : 256 * p + 256], cf2[:],
                        start=False, stop=True,
                    )
                a1 = a1pool.tile([128, 512], bf16)
                if ci == 0:
                    nc.vector.tensor_copy(a1[:], pa[:])
                else:
                    nc.scalar.copy(a1[:], pa[:])
                st["tt"], st["a1"] = tt, a1

            def fwd_s2(ci, st):
                """stage2 + pointwise -> z."""
                tt, a1 = st["tt"], st["a1"]
                px = psum.tile([128, 512], f32, tag="fwd", bufs=2)
                for p in range(NP):
                    reg = px[:, 256 * p : 256 * p + 256]
                    nc.tensor.matmul(
                        reg, a1[:, 256 * p : 256 * p + 128], cf[:],
                        start=True, stop=False,
                    )
                    nc.tensor.matmul(
                        reg, a1[:, 256 * p + 128 : 256 * p + 256], cf2[:],
                        start=False, stop=True,
                    )

                # xsb2 [g(3), p(2), q(128)]: g=0 -> s=Xr+Xi, g=1 -> Xr, g=2 -> Xi
                xsb2 = xspool.tile([128, 768], bf16)
                cast_eng = nc.vector.tensor_copy if ci == 0 else nc.scalar.copy
                cast_eng(
                    xsb2[:, 256:768].rearrange("n (c p q) -> n c p q", c=2, p=2),
                    px[:].rearrange("n (p c q) -> n c p q", p=2, c=2),
                )
                nc.vector.tensor_add(xsb2[:, 0:256], xsb2[:, 256:512], xsb2[:, 512:768])

                # Gauss products: pp [p, ab, k, q]
                pp = pppool.tile([128, 3072], bf16)
                ppv = pp[:].rearrange("n (p ab k q) -> n p ab k q", p=2, ab=4, k=3)
                ttv = tt[:].rearrange("n (ab k q) -> n ab k q", ab=4, k=3)
                xsv = xsb2[:].rearrange("n (g p q) -> n p g q", g=3, p=2)
                for p in range(NP):
                    nc.vector.tensor_mul(
                        ppv[:, p], ttv,
                        xsv[:, p].unsqueeze(1).broadcast_to((128, 4, 3, 128)),
                    )
                z = zpool.tile([128, 2048], bf16)
                nc.vector.tensor_sub(
                    z[:, 0:1024].rearrange("n (p ab q) -> n p ab q", p=2, ab=4),
                    ppv[:, :, :, 0, :],
                    ppv[:, :, :, 2, :],
                )
                nc.vector.tensor_add(
                    z[:, 1024:2048].rearrange("n (p ab q) -> n p ab q", p=2, ab=4),
                    ppv[:, :, :, 0, :],
                    ppv[:, :, :, 1, :],
                )
                st["z"] = z

            def z_slice(z, p, ab, comp):
                off = comp * 1024 + p * 512 + ab * 128
                return z[:, off : off + 128]

            def inv_A(ci, st, p):
                """stageA for pair p + bs copies (h0 on ACT, h1 on DVE)."""
                z = st["z"]
                bs = bspool.tile([128, 1024], bf16)
                bsv = bs[:].rearrange("n (c ab m) -> n c ab m", c=2, ab=4)
                for h in range(2):
                    pb = psum.tile([128, 512], f32, tag="pb", bufs=2)
                    for j in range(2):
                        ab = 2 * h + j
                        reg = pb[:, 256 * j : 256 * j + 256]
                        nc.tensor.matmul(
                            reg, z_slice(z, p, ab, 0), cg[:], start=True, stop=False
                        )
                        nc.tensor.matmul(
                            reg, z_slice(z, p, ab, 1), cg2[:], start=False, stop=True
                        )
                    src_v = pb[:].rearrange("n (ab c m) -> n c ab m", ab=2, c=2)
                    dst_v = bsv[:, :, 2 * h : 2 * h + 2, :]
                    if h == 0:
                        nc.scalar.copy(dst_v, src_v)
                    else:
                        nc.vector.tensor_copy(dst_v, src_v)
                st[f"bs{p}"] = bs

            def inv_B(ci, st, p):
                """stageB flipped + output copy + DMA."""
                bias_ap = bias_t[:, ci : ci + 1]
                bs = st[f"bs{p}"]
                mov_r = bs[:, 0:512]
                mov_i = bs[:, 512:1024]
                po = psum.tile([128, 1024], f32, tag="po", bufs=2)
                nc.tensor.matmul(po[:, 0:512], st_fr, mov_r, start=True, stop=False)
                nc.tensor.matmul(po[:, 0:512], st_fi, mov_i, start=False, stop=True)
                nc.tensor.matmul(po[:, 512:1024], st_fin, mov_r, start=True, stop=False)
                nc.tensor.matmul(po[:, 512:1024], st_fr, mov_i, start=False, stop=True)

                ot = otpool.tile([128, 1024], bf16)
                if ci == CPC - 1:
                    nc.scalar.add(ot[:, 0:512], po[:, 0:512], bias_ap)
                    nc.vector.tensor_scalar_add(
                        ot[:, 512:1024], po[:, 512:1024], bias_ap
                    )
                else:
                    nc.scalar.add(ot[:], po[:], bias_ap)
                nc.sync.dma_start(out_ext[ci, p], ot[:])

            # phase-interleaved software pipeline: forward stages of channel
            # ci+1 are woven between the inverse blocks of channel ci so every
            # cross-engine product has a full matmul-block window to land in
            states = {0: {}, 1: {}}
            fwd_s1(0, states[0])
            fwd_s2(0, states[0])
            fwd_s1(1, states[1])
            fwd_s2(1, states[1])
            for ci in range(CPC):
                nxt = ci + 2
                if nxt < CPC:
                    states[nxt] = {}
                    fwd_s1(nxt, states[nxt])
                inv_A(ci, states[ci], 0)
                if nxt < CPC:
                    fwd_s2(nxt, states[nxt])
                inv_A(ci, states[ci], 1)
                inv_B(ci, states[ci], 0)
                inv_B(ci, states[ci], 1)
                del states[ci]

    nc.finalize()
    _CACHED_NC = nc
    return nc


# ----------------------------------------------------------------------------
# public entry point
# ----------------------------------------------------------------------------
def _run(x, weight, bias, lambda_reg, trace=False, trace_kwargs=None):
    x = np.asarray(x)
    weight = np.asarray(weight)
    bias = np.asarray(bias)
    lam = float(np.asarray(lambda_reg).reshape(()))

    tc_all = _precompute_tc(weight, lam)  # [C,128,1536] bf16
    bias_vals = np.asarray(bias, np.float32).reshape(C)
    x_bf = np.asarray(x, np.float32).astype(BF16)

    in_maps = []
    for k in range(NCORES):
        c0, c1 = k * CPC, (k + 1) * CPC
        in_maps.append(
            {
                "xs": np.ascontiguousarray(
                    x_bf[:, c0:c1].transpose(1, 2, 0, 3).reshape(CPC, H, B * W)
                ),
                "tc": np.ascontiguousarray(tc_all[c0:c1]),
                "bias": np.ascontiguousarray(
                    np.broadcast_to(bias_vals[c0:c1][None, :], (128, CPC))
                ),
            }
        )

    nc = _build_nc()
    kwargs = {}
    if trace:
        kwargs["trace"] = True
        if trace_kwargs:
            kwargs.update(trace_kwargs)
    res = run_bass_kernel_spmd(nc, in_maps, list(range(NCORES)), **kwargs)

    out = np.empty((B, C, HS, WS), np.float32)
    for k in range(NCORES):
        c0, c1 = k * CPC, (k + 1) * CPC
        oc = np.asarray(res.results[k]["out"], np.float32)  # [CPC,2,128,1024]
        # [ci, p, n, t, a, b, m] -> [p, t, ci, m, a, n, b] -> [4, CPC, 256, 256]
        oc = oc.reshape(CPC, NP, 128, 2, 2, 2, 128).transpose(1, 3, 0, 6, 4, 2, 5)
        out[:, c0:c1] = oc.reshape(B, CPC, HS, WS)
    return out, res


def kernel(x, weight, bias, lambda_reg):
    out, _ = _run(x, weight, bias, lambda_reg)
    return out


# revision 28
# speedup vs baseline: 1.0288x; 1.0288x over previous
"""Converse2D (FFT-based closed-form deconvolution solve) on 8 Trainium2 cores.

v3 design:
  - Batch-packed forward FFT: x01 = x[2p] + i*x[2p+1] shares one complex
    128x128 FFT between two batches (forward matmul columns 768 -> 512
    per real slice).
  - Parity-separated inverse: Z_ab = X01 * T_ab per output parity ab;
    out[2p] = Re(ifft(Z_ab)), out[2p+1] = Im(ifft(Z_ab)).
  - stageA (row ifft) data-stationary; stageB (col ifft) flipped to
    constant-stationary form (stat = Fr/Fi/-Fi), 4 matmuls of N=512 per
    pair with a strided moving operand.
  - Software pipelining: forward+pointwise of channel ci is emitted ahead
    of the inverse of channel ci-1 so the DVE pointwise chain overlaps
    the PE's inverse matmuls.
  - T stored per parity as [Tr | Ti | Tr] so both pointwise products
    ([Tr|Ti] and [Ti|Tr]) read contiguous 256-blocks; all PSUM->SBUF
    copies have contiguous destinations.
  - bf16 output tiles (halves the output DMA; host converts to f32).

Sharding: core k handles channels [8k, 8k+8), all 4 batches (transfer
functions are reused across the batch).
"""

import numpy as np
import ml_dtypes

import concourse.bass as bass
import concourse.bacc as bacc
import concourse.mybir as mybir
import concourse.tile as tile
from concourse.bass_utils import run_bass_kernel_spmd

BF16 = ml_dtypes.bfloat16

B, C, H, W, KK = 4, 64, 128, 128, 5
S = 2
HS, WS = H * S, W * S
NCORES = 8
CPC = C // NCORES  # channels per core
NP = 2  # batch pairs per channel


# ----------------------------------------------------------------------------
# host-side precompute of the per-parity transfer functions
# ----------------------------------------------------------------------------
def _precompute_tc(weight: np.ndarray, lam: float) -> np.ndarray:
    """-> [C, 128, 1536] bf16: per parity ab=2a+b, blocks [Tr | Ti | Tr]."""
    psf = np.asarray(weight, np.float64)[0]  # [C,5,5]
    otf = np.zeros((C, HS, WS), np.complex128)
    otf[:, :KK, :KK] = psf
    otf = np.roll(otf, (-(KK // 2), -(KK // 2)), axis=(-2, -1))
    FB = np.fft.fft2(otf)
    FBC = np.conj(FB)
    F2B = (FB * FBC).real
    u = np.arange(HS)
    du = 1.0 + np.exp(-2j * np.pi * u / HS)
    Gbig = FBC + lam * du[:, None] * du[None, :]

    def quad_mean(A):
        return 0.25 * (A[:, :H, :W] + A[:, H:, :W] + A[:, :H, W:] + A[:, H:, W:])

    M = quad_mean(FB * Gbig) / (quad_mean(F2B) + lam)
    T = (Gbig - FBC * np.tile(M, (1, 2, 2))) / lam  # Tbig [C,256,256]

    ph = np.exp(2j * np.pi * np.arange(H) / HS)
    scale = 1.0 / (HS * WS)  # folds the 256-point ifft2 normalization
    blocks = []
    for a in range(2):
        for b in range(2):
            acc = np.zeros((C, H, W), np.complex128)
            for be in range(2):
                for ga in range(2):
                    acc += ((-1) ** (a * be + b * ga)) * T[
                        :, be * H : (be + 1) * H, ga * W : (ga + 1) * W
                    ]
            Tab = scale * (ph[:, None] ** a) * (ph[None, :] ** b) * acc
            Tr, Ti = Tab.real, Tab.imag
            blocks += [Tr, Ti - Tr, Ti + Tr]
    out = np.concatenate(blocks, axis=-1)  # [C,128,12*128=1536]
    return np.asarray(out, np.float32).astype(BF16)


# ----------------------------------------------------------------------------
# device program (built once, SPMD across 8 cores)
# ----------------------------------------------------------------------------
_CACHED_NC = None


def _build_nc():
    global _CACHED_NC
    if _CACHED_NC is not None:
        return _CACHED_NC

    f32 = mybir.dt.float32
    bf16 = mybir.dt.bfloat16

    idx = np.arange(H)
    Fc = np.exp(-2j * np.pi * np.outer(idx, idx) / H)
    Fr = Fc.real.astype(np.float32)
    Fi = Fc.imag.astype(np.float32)
    # forward rhs:  CF = [Fr | Fi],  CF2 = [-Fi | Fr]
    # inverse (G = conj(F) = Fr - i*Fi): CG = [Fr | -Fi], CG2 = [Fi | Fr]
    CF = np.concatenate([Fr, Fi], axis=1).astype(BF16)
    CF2 = np.concatenate([-Fi, Fr], axis=1).astype(BF16)
    CG = np.concatenate([Fr, -Fi], axis=1).astype(BF16)
    CG2 = np.concatenate([Fi, Fr], axis=1).astype(BF16)

    nc = bacc.Bacc()
    xs_ext = nc.dram_tensor("xs", [CPC, H, B * W], bf16, kind="ExternalInput")
    tc_ext = nc.dram_tensor("tc", [CPC, H, 1536], bf16, kind="ExternalInput")
    bias_ext = nc.dram_tensor("bias", [128, CPC], f32, kind="ExternalInput")
    out_ext = nc.dram_tensor("out", [CPC, NP, H, 1024], bf16, kind="ExternalOutput")

    cf_d = nc.inline_tensor(CF, "cf_d")
    cf2_d = nc.inline_tensor(CF2, "cf2_d")
    cg_d = nc.inline_tensor(CG, "cg_d")
    cg2_d = nc.inline_tensor(CG2, "cg2_d")

    with tile.TileContext(nc) as tc:
        from contextlib import ExitStack

        with ExitStack() as ctx:
            consts = ctx.enter_context(tc.tile_pool(name="consts", bufs=1))
            xpool = ctx.enter_context(tc.tile_pool(name="xpool", bufs=8))
            tpool = ctx.enter_context(tc.tile_pool(name="tpool", bufs=8))
            a1pool = ctx.enter_context(tc.tile_pool(name="a1pool", bufs=8))
            xspool = ctx.enter_context(tc.tile_pool(name="xspool", bufs=8))
            pppool = ctx.enter_context(tc.tile_pool(name="pppool", bufs=2))
            zpool = ctx.enter_context(tc.tile_pool(name="zpool", bufs=8))
            bspool = ctx.enter_context(tc.tile_pool(name="bspool", bufs=16))
            otpool = ctx.enter_context(tc.tile_pool(name="otpool", bufs=16))
            psum = ctx.enter_context(tc.tile_pool(name="psum", bufs=2, space="PSUM"))

            cf = consts.tile([128, 256], bf16, tag="cf")
            cf2 = consts.tile([128, 256], bf16, tag="cf2")
            cg = consts.tile([128, 256], bf16, tag="cg")
            cg2 = consts.tile([128, 256], bf16, tag="cg2")
            bias_t = consts.tile([128, CPC], f32, tag="bias")
            nc.sync.dma_start(cf[:], cf_d[:])
            nc.sync.dma_start(cf2[:], cf2_d[:])
            nc.sync.dma_start(cg[:], cg_d[:])
            nc.sync.dma_start(cg2[:], cg2_d[:])
            nc.sync.dma_start(bias_t[:], bias_ext[:])

            # stageB constant stationaries (slices of forward/inverse consts):
            #   Fr = cf[:,0:128], Fi = cf[:,128:256], Fin = -Fi = cg[:,128:256]
            st_fr = cf[:, 0:128]
            st_fi = cf[:, 128:256]
            st_fin = cg[:, 128:256]

            def fwd_s1(ci, st):
                """input DMAs + stage1 + a1 copy."""
                xt = xpool.tile([128, B * W], bf16)
                nc.gpsimd.dma_start(xt[:], xs_ext[ci])
                tt = tpool.tile([128, 1536], bf16)
                nc.gpsimd.dma_start(tt[:], tc_ext[ci])

                # stage1: pa[j, (p,c,u)] ; pair p packs batches 2p, 2p+1
                pa = psum.tile([128, 512], f32, tag="fwd", bufs=2)
                for p in range(NP):
                    reg = pa[:, 256 * p : 256 * p + 256]
                    nc.tensor.matmul(
                        reg, xt[:, 256 * p : 256 * p + 128], cf[:],
                        start=True, stop=False,
                    )
                    nc.tensor.matmul(
                        reg, xt[:, 256 * p + 128 : 256 * p + 256], cf2[:],
                        start=False, stop=True,
                    )
                a1 = a1pool.tile([128, 512], bf16)
                if ci == 0:
                    nc.vector.tensor_copy(a1[:], pa[:])
                else:
                    nc.scalar.copy(a1[:], pa[:])
                st["tt"], st["a1"] = tt, a1

            def fwd_s2(ci, st):
                """stage2 + pointwise -> z."""
                tt, a1 = st["tt"], st["a1"]
                px = psum.tile([128, 512], f32, tag="fwd", bufs=2)
                for p in range(NP):
                    reg = px[:, 256 * p : 256 * p + 256]
                    nc.tensor.matmul(
                        reg, a1[:, 256 * p : 256 * p + 128], cf[:],
                        start=True, stop=False,
                    )
                    nc.tensor.matmul(
                        reg, a1[:, 256 * p + 128 : 256 * p + 256], cf2[:],
                        start=False, stop=True,
                    )

                # xsb2 [g(3), p(2), q(128)]: g=0 -> s=Xr+Xi, g=1 -> Xr, g=2 -> Xi
                xsb2 = xspool.tile([128, 768], bf16)
                cast_eng = nc.vector.tensor_copy if ci == 0 else nc.scalar.copy
                cast_eng(
                    xsb2[:, 256:768].rearrange("n (c p q) -> n c p q", c=2, p=2),
                    px[:].rearrange("n (p c q) -> n c p q", p=2, c=2),
                )
                nc.vector.tensor_add(xsb2[:, 0:256], xsb2[:, 256:512], xsb2[:, 512:768])

                # Gauss products: pp [p, ab, k, q]
                pp = pppool.tile([128, 3072], bf16)
                ppv = pp[:].rearrange("n (p ab k q) -> n p ab k q", p=2, ab=4, k=3)
                ttv = tt[:].rearrange("n (ab k q) -> n ab k q", ab=4, k=3)
                xsv = xsb2[:].rearrange("n (g p q) -> n p g q", g=3, p=2)
                for p in range(NP):
                    nc.vector.tensor_mul(
                        ppv[:, p], ttv,
                        xsv[:, p].unsqueeze(1).broadcast_to((128, 4, 3, 128)),
                    )
                z = zpool.tile([128, 2048], bf16)
                nc.vector.tensor_sub(
                    z[:, 0:1024].rearrange("n (p ab q) -> n p ab q", p=2, ab=4),
                    ppv[:, :, :, 0, :],
                    ppv[:, :, :, 2, :],
                )
                nc.vector.tensor_add(
                    z[:, 1024:2048].rearrange("n (p ab q) -> n p ab q", p=2, ab=4),
                    ppv[:, :, :, 0, :],
                    ppv[:, :, :, 1, :],
                )
                st["z"] = z

            def z_slice(z, p, ab, comp):
                off = comp * 1024 + p * 512 + ab * 128
                return z[:, off : off + 128]

            def inv_A(ci, st, p):
                """stageA for pair p + bs copies (h0 on ACT, h1 on DVE)."""
                z = st["z"]
                bs = bspool.tile([128, 1024], bf16)
                bsv = bs[:].rearrange("n (c ab m) -> n c ab m", c=2, ab=4)
                for h in range(2):
                    pb = psum.tile([128, 512], f32, tag="pb", bufs=2)
                    for j in range(2):
                        ab = 2 * h + j
                        reg = pb[:, 256 * j : 256 * j + 256]
                        nc.tensor.matmul(
                            reg, z_slice(z, p, ab, 0), cg[:], start=True, stop=False
                        )
                        nc.tensor.matmul(
                            reg, z_slice(z, p, ab, 1), cg2[:], start=False, stop=True
                        )
                    src_v = pb[:].rearrange("n (ab c m) -> n c ab m", ab=2, c=2)
                    dst_v = bsv[:, :, 2 * h : 2 * h + 2, :]
                    if h == 0:
                        nc.scalar.copy(dst_v, src_v)
                    else:
                        nc.vector.tensor_copy(dst_v, src_v)
                st[f"bs{p}"] = bs

            def inv_B(ci, st, p):
                """stageB flipped + output copy + DMA."""
                bias_ap = bias_t[:, ci : ci + 1]
                bs = st[f"bs{p}"]
                mov_r = bs[:, 0:512]
                mov_i = bs[:, 512:1024]
                po = psum.tile([128, 1024], f32, tag="po", bufs=2)
                nc.tensor.matmul(po[:, 0:512], st_fr, mov_r, start=True, stop=False)
                nc.tensor.matmul(po[:, 0:512], st_fi, mov_i, start=False, stop=True)
                nc.tensor.matmul(po[:, 512:1024], st_fin, mov_r, start=True, stop=False)
                nc.tensor.matmul(po[:, 512:1024], st_fr, mov_i, start=False, stop=True)

                ot = otpool.tile([128, 1024], bf16)
                if ci == CPC - 1:
                    nc.scalar.add(ot[:, 0:512], po[:, 0:512], bias_ap)
                    nc.vector.tensor_scalar_add(
                        ot[:, 512:1024], po[:, 512:1024], bias_ap
                    )
                else:
                    nc.scalar.add(ot[:], po[:], bias_ap)
                nc.sync.dma_start(out_ext[ci, p], ot[:])

            # phase-interleaved software pipeline: forward stages of channel
            # ci+1 are woven between the inverse blocks of channel ci so every
            # cross-engine product has a full matmul-block window to land in
            states = {0: {}, 1: {}}
            fwd_s1(0, states[0])
            fwd_s2(0, states[0])
            fwd_s1(1, states[1])
            fwd_s2(1, states[1])
            for ci in range(CPC):
                nxt = ci + 2
                if nxt < CPC:
                    states[nxt] = {}
                    fwd_s1(nxt, states[nxt])
                inv_A(ci, states[ci], 0)
                if nxt < CPC:
                    fwd_s2(nxt, states[nxt])
                inv_A(ci, states[ci], 1)
                inv_B(ci, states[ci], 0)
                inv_B(ci, states[ci], 1)
                del states[ci]

    nc.finalize()
    _CACHED_NC = nc
    return nc


# ----------------------------------------------------------------------------
# public entry point
# ----------------------------------------------------------------------------
def _run(x, weight, bias, lambda_reg, trace=False, trace_kwargs=None):
    x = np.asarray(x)
    weight = np.asarray(weight)
    bias = np.asarray(bias)
    lam = float(np.asarray(lambda_reg).reshape(()))

    tc_all = _precompute_tc(weight, lam)  # [C,128,1536] bf16
    bias_vals = np.asarray(bias, np.float32).reshape(C)
    x_bf = np.asarray(x, np.float32).astype(BF16)

    in_maps = []
    for k in range(NCORES):
        c0, c1 = k * CPC, (k + 1) * CPC
        in_maps.append(
            {
                "xs": np.ascontiguousarray(
                    x_bf[:, c0:c1].transpose(1, 2, 0, 3).reshape(CPC, H, B * W)
                ),
                "tc": np.ascontiguousarray(tc_all[c0:c1]),
                "bias": np.ascontiguousarray(
                    np.broadcast_to(bias_vals[c0:c1][None, :], (128, CPC))
                ),
            }
        )

    nc = _build_nc()
    kwargs = {}
    if trace:
        kwargs["trace"] = True
        if trace_kwargs:
            kwargs.update(trace_kwargs)
    res = run_bass_kernel_spmd(nc, in_maps, list(range(NCORES)), **kwargs)

    out = np.empty((B, C, HS, WS), np.float32)
    for k in range(NCORES):
        c0, c1 = k * CPC, (k + 1) * CPC
        oc = np.asarray(res.results[k]["out"], np.float32)  # [CPC,2,128,1024]
        # [ci, p, n, t, a, b, m] -> [p, t, ci, m, a, n, b] -> [4, CPC, 256, 256]
        oc = oc.reshape(CPC, NP, 128, 2, 2, 2, 128).transpose(1, 3, 0, 6, 4, 2, 5)
        out[:, c0:c1] = oc.reshape(B, CPC, HS, WS)
    return out, res


def kernel(x, weight, bias, lambda_reg):
    out, _ = _run(x, weight, bias, lambda_reg)
    return out


# revision 29
# speedup vs baseline: 1.0298x; 1.0010x over previous
"""Converse2D (FFT-based closed-form deconvolution solve) on 8 Trainium2 cores.

v3 design:
  - Batch-packed forward FFT: x01 = x[2p] + i*x[2p+1] shares one complex
    128x128 FFT between two batches (forward matmul columns 768 -> 512
    per real slice).
  - Parity-separated inverse: Z_ab = X01 * T_ab per output parity ab;
    out[2p] = Re(ifft(Z_ab)), out[2p+1] = Im(ifft(Z_ab)).
  - stageA (row ifft) data-stationary; stageB (col ifft) flipped to
    constant-stationary form (stat = Fr/Fi/-Fi), 4 matmuls of N=512 per
    pair with a strided moving operand.
  - Software pipelining: forward+pointwise of channel ci is emitted ahead
    of the inverse of channel ci-1 so the DVE pointwise chain overlaps
    the PE's inverse matmuls.
  - T stored per parity as [Tr | Ti | Tr] so both pointwise products
    ([Tr|Ti] and [Ti|Tr]) read contiguous 256-blocks; all PSUM->SBUF
    copies have contiguous destinations.
  - bf16 output tiles (halves the output DMA; host converts to f32).

Sharding: core k handles channels [8k, 8k+8), all 4 batches (transfer
functions are reused across the batch).
"""

import numpy as np
import ml_dtypes

import concourse.bass as bass
import concourse.bacc as bacc
import concourse.mybir as mybir
import concourse.tile as tile
from concourse.bass_utils import run_bass_kernel_spmd

BF16 = ml_dtypes.bfloat16

B, C, H, W, KK = 4, 64, 128, 128, 5
S = 2
HS, WS = H * S, W * S
NCORES = 8
CPC = C // NCORES  # channels per core
NP = 2  # batch pairs per channel


# ----------------------------------------------------------------------------
# host-side precompute of the per-parity transfer functions
# ----------------------------------------------------------------------------
def _precompute_tc(weight: np.ndarray, lam: float) -> np.ndarray:
    """-> [C, 128, 1536] bf16: per parity ab=2a+b, blocks [Tr | Ti | Tr]."""
    psf = np.asarray(weight, np.float64)[0]  # [C,5,5]
    otf = np.zeros((C, HS, WS), np.complex128)
    otf[:, :KK, :KK] = psf
    otf = np.roll(otf, (-(KK // 2), -(KK // 2)), axis=(-2, -1))
    FB = np.fft.fft2(otf)
    FBC = np.conj(FB)
    F2B = (FB * FBC).real
    u = np.arange(HS)
    du = 1.0 + np.exp(-2j * np.pi * u / HS)
    Gbig = FBC + lam * du[:, None] * du[None, :]

    def quad_mean(A):
        return 0.25 * (A[:, :H, :W] + A[:, H:, :W] + A[:, :H, W:] + A[:, H:, W:])

    M = quad_mean(FB * Gbig) / (quad_mean(F2B) + lam)
    T = (Gbig - FBC * np.tile(M, (1, 2, 2))) / lam  # Tbig [C,256,256]

    ph = np.exp(2j * np.pi * np.arange(H) / HS)
    scale = 1.0 / (HS * WS)  # folds the 256-point ifft2 normalization
    blocks = []
    for a in range(2):
        for b in range(2):
            acc = np.zeros((C, H, W), np.complex128)
            for be in range(2):
                for ga in range(2):
                    acc += ((-1) ** (a * be + b * ga)) * T[
                        :, be * H : (be + 1) * H, ga * W : (ga + 1) * W
                    ]
            Tab = scale * (ph[:, None] ** a) * (ph[None, :] ** b) * acc
            Tr, Ti = Tab.real, Tab.imag
            blocks += [Tr, Ti - Tr, Ti + Tr]
    out = np.concatenate(blocks, axis=-1)  # [C,128,12*128=1536]
    return np.asarray(out, np.float32).astype(BF16)


# ----------------------------------------------------------------------------
# device program (built once, SPMD across 8 cores)
# ----------------------------------------------------------------------------
_CACHED_NC = None


def _build_nc():
    global _CACHED_NC
    if _CACHED_NC is not None:
        return _CACHED_NC

    f32 = mybir.dt.float32
    bf16 = mybir.dt.bfloat16

    idx = np.arange(H)
    Fc = np.exp(-2j * np.pi * np.outer(idx, idx) / H)
    Fr = Fc.real.astype(np.float32)
    Fi = Fc.imag.astype(np.float32)
    # forward rhs:  CF = [Fr | Fi],  CF2 = [-Fi | Fr]
    # inverse (G = conj(F) = Fr - i*Fi): CG = [Fr | -Fi], CG2 = [Fi | Fr]
    CF = np.concatenate([Fr, Fi], axis=1).astype(BF16)
    CF2 = np.concatenate([-Fi, Fr], axis=1).astype(BF16)
    CG = np.concatenate([Fr, -Fi], axis=1).astype(BF16)
    CG2 = np.concatenate([Fi, Fr], axis=1).astype(BF16)

    nc = bacc.Bacc()
    xs_ext = nc.dram_tensor("xs", [CPC, H, B * W], bf16, kind="ExternalInput")
    tc_ext = nc.dram_tensor("tc", [CPC, H, 1536], bf16, kind="ExternalInput")
    bias_ext = nc.dram_tensor("bias", [128, CPC], f32, kind="ExternalInput")
    out_ext = nc.dram_tensor("out", [CPC, NP, H, 1024], bf16, kind="ExternalOutput")

    cf_d = nc.inline_tensor(CF, "cf_d")
    cf2_d = nc.inline_tensor(CF2, "cf2_d")
    cg_d = nc.inline_tensor(CG, "cg_d")
    cg2_d = nc.inline_tensor(CG2, "cg2_d")

    with tile.TileContext(nc) as tc:
        from contextlib import ExitStack

        with ExitStack() as ctx:
            consts = ctx.enter_context(tc.tile_pool(name="consts", bufs=1))
            xpool = ctx.enter_context(tc.tile_pool(name="xpool", bufs=8))
            tpool = ctx.enter_context(tc.tile_pool(name="tpool", bufs=8))
            a1pool = ctx.enter_context(tc.tile_pool(name="a1pool", bufs=8))
            xspool = ctx.enter_context(tc.tile_pool(name="xspool", bufs=8))
            pppool = ctx.enter_context(tc.tile_pool(name="pppool", bufs=2))
            zpool = ctx.enter_context(tc.tile_pool(name="zpool", bufs=8))
            bspool = ctx.enter_context(tc.tile_pool(name="bspool", bufs=16))
            otpool = ctx.enter_context(tc.tile_pool(name="otpool", bufs=16))
            psum = ctx.enter_context(tc.tile_pool(name="psum", bufs=2, space="PSUM"))

            cf = consts.tile([128, 256], bf16, tag="cf")
            cf2 = consts.tile([128, 256], bf16, tag="cf2")
            cg = consts.tile([128, 256], bf16, tag="cg")
            cg2 = consts.tile([128, 256], bf16, tag="cg2")
            bias_t = consts.tile([128, CPC], f32, tag="bias")
            nc.sync.dma_start(cf[:], cf_d[:])
            nc.sync.dma_start(cf2[:], cf2_d[:])
            nc.sync.dma_start(cg[:], cg_d[:])
            nc.sync.dma_start(cg2[:], cg2_d[:])
            nc.sync.dma_start(bias_t[:], bias_ext[:])

            # stageB constant stationaries (slices of forward/inverse consts):
            #   Fr = cf[:,0:128], Fi = cf[:,128:256], Fin = -Fi = cg[:,128:256]
            st_fr = cf[:, 0:128]
            st_fi = cf[:, 128:256]
            st_fin = cg[:, 128:256]

            def fwd_s1(ci, st):
                """input DMAs + stage1 + a1 copy."""
                xt = xpool.tile([128, B * W], bf16)
                nc.gpsimd.dma_start(xt[:], xs_ext[ci])
                tt = tpool.tile([128, 1536], bf16)
                nc.gpsimd.dma_start(tt[:], tc_ext[ci])

                # stage1: pa[j, (p,c,u)] ; pair p packs batches 2p, 2p+1
                pa = psum.tile([128, 512], f32, tag="fwd", bufs=2)
                for p in range(NP):
                    reg = pa[:, 256 * p : 256 * p + 256]
                    nc.tensor.matmul(
                        reg, xt[:, 256 * p : 256 * p + 128], cf[:],
                        start=True, stop=False,
                    )
                    nc.tensor.matmul(
                        reg, xt[:, 256 * p + 128 : 256 * p + 256], cf2[:],
                        start=False, stop=True,
                    )
                a1 = a1pool.tile([128, 512], bf16)
                if ci == 0:
                    nc.vector.tensor_copy(a1[:], pa[:])
                else:
                    nc.scalar.copy(a1[:], pa[:])
                st["tt"], st["a1"] = tt, a1

            def fwd_s2(ci, st):
                """stage2 + pointwise -> z."""
                tt, a1 = st["tt"], st["a1"]
                px = psum.tile([128, 512], f32, tag="fwd", bufs=2)
                for p in range(NP):
                    reg = px[:, 256 * p : 256 * p + 256]
                    nc.tensor.matmul(
                        reg, a1[:, 256 * p : 256 * p + 128], cf[:],
                        start=True, stop=False,
                    )
                    nc.tensor.matmul(
                        reg, a1[:, 256 * p + 128 : 256 * p + 256], cf2[:],
                        start=False, stop=True,
                    )

                # xsb2 [g(3), p(2), q(128)]: g=0 -> s=Xr+Xi, g=1 -> Xr, g=2 -> Xi
                xsb2 = xspool.tile([128, 768], bf16)
                cast_eng = nc.vector.tensor_copy if ci == 0 else nc.scalar.copy
                cast_eng(
                    xsb2[:, 256:768].rearrange("n (c p q) -> n c p q", c=2, p=2),
                    px[:].rearrange("n (p c q) -> n c p q", p=2, c=2),
                )
                nc.vector.tensor_add(xsb2[:, 0:256], xsb2[:, 256:512], xsb2[:, 512:768])

                # Gauss products: pp [p, ab, k, q]
                pp = pppool.tile([128, 3072], bf16)
                ppv = pp[:].rearrange("n (p ab k q) -> n p ab k q", p=2, ab=4, k=3)
                ttv = tt[:].rearrange("n (ab k q) -> n ab k q", ab=4, k=3)
                xsv = xsb2[:].rearrange("n (g p q) -> n p g q", g=3, p=2)
                for p in range(NP):
                    nc.vector.tensor_mul(
                        ppv[:, p], ttv,
                        xsv[:, p].unsqueeze(1).broadcast_to((128, 4, 3, 128)),
                    )
                z = zpool.tile([128, 2048], bf16)
                nc.vector.tensor_sub(
                    z[:, 0:1024].rearrange("n (p ab q) -> n p ab q", p=2, ab=4),
                    ppv[:, :, :, 0, :],
                    ppv[:, :, :, 2, :],
                )
                nc.vector.tensor_add(
                    z[:, 1024:2048].rearrange("n (p ab q) -> n p ab q", p=2, ab=4),
                    ppv[:, :, :, 0, :],
                    ppv[:, :, :, 1, :],
                )
                st["z"] = z

            def z_slice(z, p, ab, comp):
                off = comp * 1024 + p * 512 + ab * 128
                return z[:, off : off + 128]

            def inv_A(ci, st, p):
                """stageA for pair p + bs copies (h0 on ACT, h1 on DVE)."""
                z = st["z"]
                bs = bspool.tile([128, 1024], bf16)
                bsv = bs[:].rearrange("n (c ab m) -> n c ab m", c=2, ab=4)
                for h in range(2):
                    pb = psum.tile([128, 512], f32, tag="pb", bufs=2)
                    for j in range(2):
                        ab = 2 * h + j
                        reg = pb[:, 256 * j : 256 * j + 256]
                        nc.tensor.matmul(
                            reg, z_slice(z, p, ab, 0), cg[:], start=True, stop=False
                        )
                        nc.tensor.matmul(
                            reg, z_slice(z, p, ab, 1), cg2[:], start=False, stop=True
                        )
                    src_v = pb[:].rearrange("n (ab c m) -> n c ab m", ab=2, c=2)
                    dst_v = bsv[:, :, 2 * h : 2 * h + 2, :]
                    if h == 0:
                        nc.scalar.copy(dst_v, src_v)
                    else:
                        nc.vector.tensor_copy(dst_v, src_v)
                st[f"bs{p}"] = bs

            def inv_B(ci, st, p):
                """stageB flipped + output copy + DMA."""
                bias_ap = bias_t[:, ci : ci + 1]
                bs = st[f"bs{p}"]
                mov_r = bs[:, 0:512]
                mov_i = bs[:, 512:1024]
                po = psum.tile([128, 1024], f32, tag="po", bufs=2)
                nc.tensor.matmul(po[:, 0:512], st_fr, mov_r, start=True, stop=False)
                nc.tensor.matmul(po[:, 0:512], st_fi, mov_i, start=False, stop=True)
                nc.tensor.matmul(po[:, 512:1024], st_fin, mov_r, start=True, stop=False)
                nc.tensor.matmul(po[:, 512:1024], st_fr, mov_i, start=False, stop=True)

                ot = otpool.tile([128, 1024], bf16)
                if ci == CPC - 1:
                    nc.scalar.add(ot[:, 0:512], po[:, 0:512], bias_ap)
                    nc.vector.tensor_scalar_add(
                        ot[:, 512:1024], po[:, 512:1024], bias_ap
                    )
                else:
                    nc.scalar.add(ot[:], po[:], bias_ap)
                nc.sync.dma_start(out_ext[ci, p], ot[:])

            # phase-interleaved software pipeline: forward stages of channel
            # ci+1 are woven between the inverse blocks of channel ci so every
            # cross-engine product has a full matmul-block window to land in
            states = {0: {}, 1: {}}
            fwd_s1(0, states[0])
            fwd_s2(0, states[0])
            fwd_s1(1, states[1])
            fwd_s2(1, states[1])
            for ci in range(CPC - 2):
                nxt = ci + 2
                states[nxt] = {}
                fwd_s1(nxt, states[nxt])
                inv_A(ci, states[ci], 0)
                fwd_s2(nxt, states[nxt])
                inv_A(ci, states[ci], 1)
                inv_B(ci, states[ci], 0)
                inv_B(ci, states[ci], 1)
                del states[ci]
            # drain: interleave the last two channels' inverse blocks so the
            # bs/out copies land inside other matmul-block windows
            c6, c7 = CPC - 2, CPC - 1
            inv_A(c6, states[c6], 0)
            inv_A(c6, states[c6], 1)
            inv_A(c7, states[c7], 0)
            inv_B(c6, states[c6], 0)
            inv_A(c7, states[c7], 1)
            inv_B(c6, states[c6], 1)
            inv_B(c7, states[c7], 0)
            inv_B(c7, states[c7], 1)

    nc.finalize()
    _CACHED_NC = nc
    return nc


# ----------------------------------------------------------------------------
# public entry point
# ----------------------------------------------------------------------------
def _run(x, weight, bias, lambda_reg, trace=False, trace_kwargs=None):
    x = np.asarray(x)
    weight = np.asarray(weight)
    bias = np.asarray(bias)
    lam = float(np.asarray(lambda_reg).reshape(()))

    tc_all = _precompute_tc(weight, lam)  # [C,128,1536] bf16
    bias_vals = np.asarray(bias, np.float32).reshape(C)
    x_bf = np.asarray(x, np.float32).astype(BF16)

    in_maps = []
    for k in range(NCORES):
        c0, c1 = k * CPC, (k + 1) * CPC
        in_maps.append(
            {
                "xs": np.ascontiguousarray(
                    x_bf[:, c0:c1].transpose(1, 2, 0, 3).reshape(CPC, H, B * W)
                ),
                "tc": np.ascontiguousarray(tc_all[c0:c1]),
                "bias": np.ascontiguousarray(
                    np.broadcast_to(bias_vals[c0:c1][None, :], (128, CPC))
                ),
            }
        )

    nc = _build_nc()
    kwargs = {}
    if trace:
        kwargs["trace"] = True
        if trace_kwargs:
            kwargs.update(trace_kwargs)
    res = run_bass_kernel_spmd(nc, in_maps, list(range(NCORES)), **kwargs)

    out = np.empty((B, C, HS, WS), np.float32)
    for k in range(NCORES):
        c0, c1 = k * CPC, (k + 1) * CPC
        oc = np.asarray(res.results[k]["out"], np.float32)  # [CPC,2,128,1024]
        # [ci, p, n, t, a, b, m] -> [p, t, ci, m, a, n, b] -> [4, CPC, 256, 256]
        oc = oc.reshape(CPC, NP, 128, 2, 2, 2, 128).transpose(1, 3, 0, 6, 4, 2, 5)
        out[:, c0:c1] = oc.reshape(B, CPC, HS, WS)
    return out, res


def kernel(x, weight, bias, lambda_reg):
    out, _ = _run(x, weight, bias, lambda_reg)
    return out
